# revision 4
# baseline (speedup 1.0000x reference)
"""GCN v4.1: dense fp8 aggregation matmul, no gather, hidden epilogue.

v4 replaced the v3 dedup-gather + fp16 multi-hot scatter stream (~103MB/core
+ 167us GpSimd gather) with a dense per-core count matrix S8 [20480 src,
2560 dst] in fp8 (counts are small ints - exact in e4m3). aggT accumulates
as sum over 256-row chunk-pairs of h8_chunk^T @ S8_chunk using fp8
DoubleRow matmuls (0.5 cyc/row). recip(deg) is applied exactly in fp32 at
PSUM->SBUF copy; epilogue GEMMs run in bf16.

v4.1 on top: column-split the accumulation into phase A (dst windows 0-3)
and phase B (window 4) so phase A's epilogue overlaps phase B's S stream;
recip is a [1,2560] row DMA-broadcast across partitions; output is fp16;
epilogue output DMAs issue from the (idle) GpSimd queue so they don't
stall the sync-engine S-stream doorbells. ~56MB/core HBM traffic.
"""

import numpy as np

N_NODES = 20000
D = 128
N_CORES = 8
N_PAD = 20480
NPC = N_PAD // N_CORES            # 2560 dst slots per core
TILE2 = 512
NCP = N_PAD // 256                # 80 src chunk-pairs (256 rows each)
COLS_A = 4 * TILE2                # windows 0-3
COLS_B = NPC - COLS_A             # window 4
BCP = 4                           # chunk-pairs per phase-B DMA tile

_prog_cache = {}


def _build_program41():
    import concourse.mybir as mybir
    from concourse import bacc
    from concourse.tile import TileContext

    dt = mybir.dt
    DR = mybir.MatmulPerfMode.DoubleRow
    nc = bacc.Bacc()

    h8 = nc.declare_dram_parameter("h8", [128, N_PAD], dt.float8e4, isOutput=False)
    smatA = nc.declare_dram_parameter(
        "smatA", [128, NCP * 2 * COLS_A], dt.float8e4, isOutput=False
    )
    smatB = nc.declare_dram_parameter(
        "smatB", [128, NCP * 2 * COLS_B], dt.float8e4, isOutput=False
    )
    hT = nc.declare_dram_parameter("hT", [D, NPC], dt.bfloat16, isOutput=False)
    recip = nc.declare_dram_parameter("recip", [1, NPC], dt.float32, isOutput=False)
    wselfT = nc.declare_dram_parameter("wselfT", [D, D], dt.bfloat16, isOutput=False)
    wneiT = nc.declare_dram_parameter("wneiT", [D, D], dt.bfloat16, isOutput=False)
    bself = nc.declare_dram_parameter("bself", [D, 1], dt.float32, isOutput=False)
    outT = nc.declare_dram_parameter("outT", [D, NPC], dt.float16, isOutput=True)

    with (
        TileContext(nc) as tc,
        tc.tile_pool(name="const", bufs=1) as cpool,
        tc.tile_pool(name="selA", bufs=6) as spoolA,
        tc.tile_pool(name="selB", bufs=4) as spoolB,
        tc.tile_pool(name="agg", bufs=3) as apool,
        tc.tile_pool(name="res", bufs=3) as opool,
        tc.tile_pool(name="pagg", bufs=1, space="PSUM") as pagg,
        tc.tile_pool(name="pout", bufs=2, space="PSUM") as pout,
    ):
        h8_sb = cpool.tile([128, N_PAD], dt.float8e4)
        nc.sync.dma_start(out=h8_sb[:], in_=h8[:])
        hT_sb = cpool.tile([D, NPC], dt.bfloat16)
        nc.sync.dma_start(out=hT_sb[:], in_=hT[:])
        recip_sb = cpool.tile([128, NPC], dt.float32)
        nc.sync.dma_start(out=recip_sb[:], in_=recip[:, :].to_broadcast([128, NPC]))
        wselfT_sb = cpool.tile([D, D], dt.bfloat16)
        nc.sync.dma_start(out=wselfT_sb[:], in_=wselfT[:])
        wneiT_sb = cpool.tile([D, D], dt.bfloat16)
        nc.sync.dma_start(out=wneiT_sb[:], in_=wneiT[:])
        bself_sb = cpool.tile([D, 1], dt.float32)
        nc.sync.dma_start(out=bself_sb[:], in_=bself[:])

        # [128, cp, 2, 128]: row (cp*256 + i*128 + p) of padded h, fp8
        h8r = h8_sb.rearrange("p (cp two m) -> p cp two m", two=2, m=128)

        def epilogue(k, pa_tile, base):
            sl = slice(k * TILE2, (k + 1) * TILE2)
            lo = k * TILE2 - base
            aggT = apool.tile([128, TILE2], dt.bfloat16)
            nc.vector.tensor_mul(
                out=aggT[:], in0=pa_tile[:, lo : lo + TILE2], in1=recip_sb[:, sl]
            )
            po = pout.tile([128, TILE2], dt.float32, space="PSUM")
            nc.tensor.matmul(
                out=po[:], lhsT=wselfT_sb[:], rhs=hT_sb[:, sl], start=True, stop=False
            )
            nc.tensor.matmul(
                out=po[:], lhsT=wneiT_sb[:], rhs=aggT[:], start=False, stop=True
            )
            o = opool.tile([128, TILE2], dt.float16)
            nc.scalar.activation(
                out=o[:],
                in_=po[:],
                func=mybir.ActivationFunctionType.Relu,
                bias=bself_sb[:, :1],
            )
            nc.gpsimd.dma_start(out=outT[:, sl], in_=o[:])

        # ---- phase A: dst windows 0-3 ----
        paA = pagg.tile([128, COLS_A], dt.float32)
        for cp in range(NCP):
            s = spoolA.tile([128, 2 * COLS_A], dt.float8e4)
            nc.sync.dma_start(
                out=s[:], in_=smatA[:, cp * 2 * COLS_A : (cp + 1) * 2 * COLS_A]
            )
            sr = s.rearrange("p (two n) -> p two n", two=2)
            for k in range(4):
                nc.tensor.matmul(
                    out=paA[:, k * TILE2 : (k + 1) * TILE2],
                    lhsT=h8r[:, cp, :, :],
                    rhs=sr[:, :, k * TILE2 : (k + 1) * TILE2],
                    start=(cp == 0),
                    stop=(cp == NCP - 1),
                    perf_mode=DR,
                )

        # ---- phase B stream (window 4) + phase A epilogue overlap ----
        paB = pagg.tile([128, COLS_B], dt.float32)
        nb = NCP // BCP
        for t in range(nb):
            s = spoolB.tile([128, BCP * 2 * COLS_B], dt.float8e4)
            nc.sync.dma_start(
                out=s[:],
                in_=smatB[:, t * BCP * 2 * COLS_B : (t + 1) * BCP * 2 * COLS_B],
            )
            sr = s.rearrange("p (c two n) -> p c two n", c=BCP, two=2)
            for j in range(BCP):
                cp = t * BCP + j
                nc.tensor.matmul(
                    out=paB[:],
                    lhsT=h8r[:, cp, :, :],
                    rhs=sr[:, j, :, :],
                    start=(cp == 0),
                    stop=(cp == NCP - 1),
                    perf_mode=DR,
                )
            if t == 0:
                for k in range(4):
                    epilogue(k, paA, 0)

        epilogue(4, paB, COLS_A)

    nc.compile()
    return nc


def _host_prep(h, edge_index, deg):
    import ml_dtypes

    f8 = ml_dtypes.float8_e4m3
    bf16 = ml_dtypes.bfloat16

    src = np.asarray(edge_index[0], dtype=np.int64)
    dst = np.asarray(edge_index[1], dtype=np.int64)
    h = np.asarray(h, dtype=np.float32)
    deg = np.asarray(deg, dtype=np.float32)

    h_pad = np.zeros((N_PAD, D), np.float32)
    h_pad[:N_NODES] = h
    h8_flat = (
        h_pad.astype(f8).reshape(NCP, 2, 128, D).transpose(2, 0, 1, 3).reshape(128, -1)
    )
    h8_flat = np.ascontiguousarray(h8_flat)

    recip = np.zeros(N_PAD, np.float32)
    recip[:N_NODES] = 1.0 / np.maximum(deg, 1.0)

    lut = np.arange(256).astype(np.float32).astype(f8)

    core_of_dst = dst // NPC
    order = np.argsort(core_of_dst, kind="stable")
    src_s, dst_s = src[order], dst[order]
    bounds = np.searchsorted(core_of_dst[order], np.arange(N_CORES + 1))

    per_core = []
    for cc in range(N_CORES):
        lo, hi = bounds[cc], bounds[cc + 1]
        s_u8 = np.zeros((N_PAD, NPC), np.uint8)
        np.add.at(s_u8, (src_s[lo:hi], dst_s[lo:hi] - cc * NPC), 1)
        s8 = lut[s_u8]
        # [128, cp, 2, cols] layouts for the two column phases
        s8r = s8.reshape(NCP, 2, 128, NPC).transpose(2, 0, 1, 3)
        sA = np.ascontiguousarray(s8r[:, :, :, :COLS_A]).reshape(128, -1)
        sB = np.ascontiguousarray(s8r[:, :, :, COLS_A:]).reshape(128, -1)
        per_core.append((sA, sB))

    hT_bf = np.ascontiguousarray(h_pad.T.astype(bf16))
    return h8_flat, per_core, recip, hT_bf


def kernel(h, edge_index, deg, w_self, b_self, w_nei):
    import os

    import ml_dtypes
    from concourse.bass_utils import run_bass_kernel_spmd

    bf16 = ml_dtypes.bfloat16

    h8_flat, per_core, recip, hT_bf = _host_prep(h, edge_index, deg)

    wselfT = np.ascontiguousarray(np.asarray(w_self, dtype=np.float32).T.astype(bf16))
    wneiT = np.ascontiguousarray(np.asarray(w_nei, dtype=np.float32).T.astype(bf16))
    b_col = np.ascontiguousarray(np.asarray(b_self, dtype=np.float32).reshape(D, 1))

    in_maps = []
    for cc in range(N_CORES):
        sA, sB = per_core[cc]
        in_maps.append(
            {
                "h8": h8_flat,
                "smatA": sA,
                "smatB": sB,
                "hT": np.ascontiguousarray(hT_bf[:, cc * NPC : (cc + 1) * NPC]),
                "recip": np.ascontiguousarray(
                    recip[cc * NPC : (cc + 1) * NPC].reshape(1, NPC)
                ),
                "wselfT": wselfT,
                "wneiT": wneiT,
                "bself": b_col,
            }
        )

    if "v41" not in _prog_cache:
        _prog_cache["v41"] = _build_program41()
    nc = _prog_cache["v41"]

    trace = bool(int(os.environ.get("GCN_TRACE", "0")))
    res = run_bass_kernel_spmd(nc, in_maps, core_ids=list(range(N_CORES)), trace=trace)
    kernel.last_results = res

    outT = np.concatenate([r["outT"] for r in res.results], axis=1)
    return np.ascontiguousarray(outT[:, :N_NODES].T.astype(np.float32))


# revision 7
# speedup vs baseline: 1.1214x; 1.1214x over previous
"""GCN v4.1: dense fp8 aggregation matmul, no gather, hidden epilogue.

v4 replaced the v3 dedup-gather + fp16 multi-hot scatter stream (~103MB/core
+ 167us GpSimd gather) with a dense per-core count matrix S8 [20480 src,
2560 dst] in fp8 (counts are small ints - exact in e4m3). aggT accumulates
as sum over 256-row chunk-pairs of h8_chunk^T @ S8_chunk using fp8
DoubleRow matmuls (0.5 cyc/row). recip(deg) is applied exactly in fp32 at
PSUM->SBUF copy; epilogue GEMMs run in bf16.

v4.1 on top: column-split the accumulation into phase A (dst windows 0-3)
and phase B (window 4) so phase A's epilogue overlaps phase B's S stream;
recip is a [1,2560] row DMA-broadcast across partitions; output is fp16;
epilogue output DMAs issue from the (idle) GpSimd queue so they don't
stall the sync-engine S-stream doorbells. ~56MB/core HBM traffic.

v4.2: fix the v4.1 regression - phase A's S tiles were [cp][2][2048]
whose DoubleRow subtile stride (2048B) lands both ifmap streams in the
same SBUF bank, halving matmul rate (427ns vs 216ns per 512-col MM) and
making phase A tensor-bound. Re-lay phase A as [cp][k][2][512] (512B
subtile stride, conflict-free, contiguous 4KB/partition DMA tiles).
"""

import numpy as np

N_NODES = 20000
D = 128
N_CORES = 8
N_PAD = 20480
NPC = N_PAD // N_CORES            # 2560 dst slots per core
TILE2 = 512
NCP = N_PAD // 256                # 80 src chunk-pairs (256 rows each)
COLS_A = 4 * TILE2                # windows 0-3
COLS_B = NPC - COLS_A             # window 4
BCP = 4                           # chunk-pairs per phase-B DMA tile

_prog_cache = {}


def _build_program41():
    import concourse.mybir as mybir
    from concourse import bacc
    from concourse.tile import TileContext

    dt = mybir.dt
    DR = mybir.MatmulPerfMode.DoubleRow
    nc = bacc.Bacc()

    h8 = nc.declare_dram_parameter("h8", [128, N_PAD], dt.float8e4, isOutput=False)
    smatA = nc.declare_dram_parameter(
        "smatA", [128, NCP * 2 * COLS_A], dt.float8e4, isOutput=False
    )
    smatB = nc.declare_dram_parameter(
        "smatB", [128, NCP * 2 * COLS_B], dt.float8e4, isOutput=False
    )
    hT = nc.declare_dram_parameter("hT", [D, NPC], dt.bfloat16, isOutput=False)
    recip = nc.declare_dram_parameter("recip", [1, NPC], dt.float32, isOutput=False)
    wselfT = nc.declare_dram_parameter("wselfT", [D, D], dt.bfloat16, isOutput=False)
    wneiT = nc.declare_dram_parameter("wneiT", [D, D], dt.bfloat16, isOutput=False)
    bself = nc.declare_dram_parameter("bself", [D, 1], dt.float32, isOutput=False)
    outT = nc.declare_dram_parameter("outT", [D, NPC], dt.float16, isOutput=True)

    with (
        TileContext(nc) as tc,
        tc.tile_pool(name="const", bufs=1) as cpool,
        tc.tile_pool(name="selA", bufs=6) as spoolA,
        tc.tile_pool(name="selB", bufs=4) as spoolB,
        tc.tile_pool(name="agg", bufs=3) as apool,
        tc.tile_pool(name="res", bufs=3) as opool,
        tc.tile_pool(name="pagg", bufs=1, space="PSUM") as pagg,
        tc.tile_pool(name="pout", bufs=2, space="PSUM") as pout,
    ):
        h8_sb = cpool.tile([128, N_PAD], dt.float8e4)
        nc.sync.dma_start(out=h8_sb[:], in_=h8[:])
        hT_sb = cpool.tile([D, NPC], dt.bfloat16)
        nc.sync.dma_start(out=hT_sb[:], in_=hT[:])
        recip_sb = cpool.tile([128, NPC], dt.float32)
        nc.sync.dma_start(out=recip_sb[:], in_=recip[:, :].to_broadcast([128, NPC]))
        wselfT_sb = cpool.tile([D, D], dt.bfloat16)
        nc.sync.dma_start(out=wselfT_sb[:], in_=wselfT[:])
        wneiT_sb = cpool.tile([D, D], dt.bfloat16)
        nc.sync.dma_start(out=wneiT_sb[:], in_=wneiT[:])
        bself_sb = cpool.tile([D, 1], dt.float32)
        nc.sync.dma_start(out=bself_sb[:], in_=bself[:])

        # [128, cp, 2, 128]: row (cp*256 + i*128 + p) of padded h, fp8
        h8r = h8_sb.rearrange("p (cp two m) -> p cp two m", two=2, m=128)

        def epilogue(k, pa_tile, base):
            sl = slice(k * TILE2, (k + 1) * TILE2)
            lo = k * TILE2 - base
            aggT = apool.tile([128, TILE2], dt.bfloat16)
            nc.vector.tensor_mul(
                out=aggT[:], in0=pa_tile[:, lo : lo + TILE2], in1=recip_sb[:, sl]
            )
            po = pout.tile([128, TILE2], dt.float32, space="PSUM")
            nc.tensor.matmul(
                out=po[:], lhsT=wselfT_sb[:], rhs=hT_sb[:, sl], start=True, stop=False
            )
            nc.tensor.matmul(
                out=po[:], lhsT=wneiT_sb[:], rhs=aggT[:], start=False, stop=True
            )
            o = opool.tile([128, TILE2], dt.float16)
            nc.scalar.activation(
                out=o[:],
                in_=po[:],
                func=mybir.ActivationFunctionType.Relu,
                bias=bself_sb[:, :1],
            )
            nc.gpsimd.dma_start(out=outT[:, sl], in_=o[:])

        # ---- phase A: dst windows 0-3 ----
        # layout [cp][k][two][512]: 512B DoubleRow subtile stride (no SBUF
        # bank conflict), contiguous 4KB/partition DMA tiles
        paA = pagg.tile([128, COLS_A], dt.float32)
        for cp in range(NCP):
            s = spoolA.tile([128, 2 * COLS_A], dt.float8e4)
            nc.sync.dma_start(
                out=s[:], in_=smatA[:, cp * 2 * COLS_A : (cp + 1) * 2 * COLS_A]
            )
            sr = s.rearrange("p (k two n) -> p k two n", k=4, two=2)
            for k in range(4):
                nc.tensor.matmul(
                    out=paA[:, k * TILE2 : (k + 1) * TILE2],
                    lhsT=h8r[:, cp, :, :],
                    rhs=sr[:, k, :, :],
                    start=(cp == 0),
                    stop=(cp == NCP - 1),
                    perf_mode=DR,
                )

        # ---- phase B stream (window 4) + phase A epilogue overlap ----
        paB = pagg.tile([128, COLS_B], dt.float32)
        nb = NCP // BCP
        for t in range(nb):
            s = spoolB.tile([128, BCP * 2 * COLS_B], dt.float8e4)
            nc.sync.dma_start(
                out=s[:],
                in_=smatB[:, t * BCP * 2 * COLS_B : (t + 1) * BCP * 2 * COLS_B],
            )
            sr = s.rearrange("p (c two n) -> p c two n", c=BCP, two=2)
            for j in range(BCP):
                cp = t * BCP + j
                nc.tensor.matmul(
                    out=paB[:],
                    lhsT=h8r[:, cp, :, :],
                    rhs=sr[:, j, :, :],
                    start=(cp == 0),
                    stop=(cp == NCP - 1),
                    perf_mode=DR,
                )
            if t == 0:
                for k in range(4):
                    epilogue(k, paA, 0)

        epilogue(4, paB, COLS_A)

    nc.compile()
    return nc


def _host_prep(h, edge_index, deg):
    import ml_dtypes

    f8 = ml_dtypes.float8_e4m3
    bf16 = ml_dtypes.bfloat16

    src = np.asarray(edge_index[0], dtype=np.int64)
    dst = np.asarray(edge_index[1], dtype=np.int64)
    h = np.asarray(h, dtype=np.float32)
    deg = np.asarray(deg, dtype=np.float32)

    h_pad = np.zeros((N_PAD, D), np.float32)
    h_pad[:N_NODES] = h
    h8_flat = (
        h_pad.astype(f8).reshape(NCP, 2, 128, D).transpose(2, 0, 1, 3).reshape(128, -1)
    )
    h8_flat = np.ascontiguousarray(h8_flat)

    recip = np.zeros(N_PAD, np.float32)
    recip[:N_NODES] = 1.0 / np.maximum(deg, 1.0)

    lut = np.arange(256).astype(np.float32).astype(f8)

    core_of_dst = dst // NPC
    order = np.argsort(core_of_dst, kind="stable")
    src_s, dst_s = src[order], dst[order]
    bounds = np.searchsorted(core_of_dst[order], np.arange(N_CORES + 1))

    per_core = []
    for cc in range(N_CORES):
        lo, hi = bounds[cc], bounds[cc + 1]
        s_u8 = np.zeros((N_PAD, NPC), np.uint8)
        np.add.at(s_u8, (src_s[lo:hi], dst_s[lo:hi] - cc * NPC), 1)
        s8 = lut[s_u8]
        # [128, cp, 2, cols] layouts for the two column phases
        s8r = s8.reshape(NCP, 2, 128, NPC).transpose(2, 0, 1, 3)
        # phase A: [128, cp, k, two, 512]; phase B: [128, cp, two, 512]
        sA = s8r[:, :, :, :COLS_A].reshape(128, NCP, 2, 4, TILE2)
        sA = np.ascontiguousarray(sA.transpose(0, 1, 3, 2, 4)).reshape(128, -1)
        sB = np.ascontiguousarray(s8r[:, :, :, COLS_A:]).reshape(128, -1)
        per_core.append((sA, sB))

    hT_bf = np.ascontiguousarray(h_pad.T.astype(bf16))
    return h8_flat, per_core, recip, hT_bf


def kernel(h, edge_index, deg, w_self, b_self, w_nei):
    import os

    import ml_dtypes
    from concourse.bass_utils import run_bass_kernel_spmd

    bf16 = ml_dtypes.bfloat16

    h8_flat, per_core, recip, hT_bf = _host_prep(h, edge_index, deg)

    wselfT = np.ascontiguousarray(np.asarray(w_self, dtype=np.float32).T.astype(bf16))
    wneiT = np.ascontiguousarray(np.asarray(w_nei, dtype=np.float32).T.astype(bf16))
    b_col = np.ascontiguousarray(np.asarray(b_self, dtype=np.float32).reshape(D, 1))

    in_maps = []
    for cc in range(N_CORES):
        sA, sB = per_core[cc]
        in_maps.append(
            {
                "h8": h8_flat,
                "smatA": sA,
                "smatB": sB,
                "hT": np.ascontiguousarray(hT_bf[:, cc * NPC : (cc + 1) * NPC]),
                "recip": np.ascontiguousarray(
                    recip[cc * NPC : (cc + 1) * NPC].reshape(1, NPC)
                ),
                "wselfT": wselfT,
                "wneiT": wneiT,
                "bself": b_col,
            }
        )

    if "v41" not in _prog_cache:
        _prog_cache["v41"] = _build_program41()
    nc = _prog_cache["v41"]

    trace = bool(int(os.environ.get("GCN_TRACE", "0")))
    res = run_bass_kernel_spmd(nc, in_maps, core_ids=list(range(N_CORES)), trace=trace)
    kernel.last_results = res

    outT = np.concatenate([r["outT"] for r in res.results], axis=1)
    return np.ascontiguousarray(outT[:, :N_NODES].T.astype(np.float32))


# revision 8
# speedup vs baseline: 1.1371x; 1.0140x over previous
"""GCN v4.3: dense fp8 aggregation matmul, no gather.

Replaces the v3 dedup-gather + fp16 multi-hot scatter stream (~103MB/core +
167us GpSimd gather) with a dense per-core count matrix S8 [20480 src,
2560 dst] in fp8 (counts are small ints - exact in e4m3). aggT accumulates
as sum over 256-row chunk-pairs of h8_chunk^T @ S8_chunk using fp8
DoubleRow matmuls; the 2560B subtile stride keeps the two DoubleRow ifmap
streams in different SBUF banks (2048B stride halves the matmul rate).
recip(deg) is applied exactly in fp32 at PSUM->SBUF copy time (recip is a
[1,2560] row DMA-broadcast across partitions); epilogue GEMMs run in bf16;
output is fp16. ~56MB/core HBM traffic, DMA-roofline bound.
"""

import numpy as np

N_NODES = 20000
D = 128
N_CORES = 8
N_PAD = 20480
NPC = N_PAD // N_CORES            # 2560 dst slots per core
TILE2 = 512
TPT = NPC // TILE2                # 5 psum column windows
NCP = N_PAD // 256                # 80 src chunk-pairs (256 rows each)

_prog_cache = {}


def _build_program43():
    import concourse.mybir as mybir
    from concourse import bacc
    from concourse.tile import TileContext

    dt = mybir.dt
    DR = mybir.MatmulPerfMode.DoubleRow
    nc = bacc.Bacc()

    h8 = nc.declare_dram_parameter("h8", [128, N_PAD], dt.float8e4, isOutput=False)
    smat = nc.declare_dram_parameter(
        "smat", [128, NCP * 2 * NPC], dt.float8e4, isOutput=False
    )
    hT = nc.declare_dram_parameter("hT", [D, NPC], dt.bfloat16, isOutput=False)
    recip = nc.declare_dram_parameter("recip", [1, NPC], dt.float32, isOutput=False)
    wselfT = nc.declare_dram_parameter("wselfT", [D, D], dt.bfloat16, isOutput=False)
    wneiT = nc.declare_dram_parameter("wneiT", [D, D], dt.bfloat16, isOutput=False)
    bself = nc.declare_dram_parameter("bself", [D, 1], dt.float32, isOutput=False)
    outT = nc.declare_dram_parameter("outT", [D, NPC], dt.float16, isOutput=True)

    with (
        TileContext(nc) as tc,
        tc.tile_pool(name="const", bufs=1) as cpool,
        tc.tile_pool(name="sel", bufs=6) as spool,
        tc.tile_pool(name="agg", bufs=3) as apool,
        tc.tile_pool(name="res", bufs=3) as opool,
        tc.tile_pool(name="pagg", bufs=1, space="PSUM") as pagg,
        tc.tile_pool(name="pout", bufs=2, space="PSUM") as pout,
    ):
        h8_sb = cpool.tile([128, N_PAD], dt.float8e4)
        nc.sync.dma_start(out=h8_sb[:], in_=h8[:])
        hT_sb = cpool.tile([D, NPC], dt.bfloat16)
        nc.sync.dma_start(out=hT_sb[:], in_=hT[:])
        recip_sb = cpool.tile([128, NPC], dt.float32)
        nc.sync.dma_start(out=recip_sb[:], in_=recip[:, :].to_broadcast([128, NPC]))
        wselfT_sb = cpool.tile([D, D], dt.bfloat16)
        nc.sync.dma_start(out=wselfT_sb[:], in_=wselfT[:])
        wneiT_sb = cpool.tile([D, D], dt.bfloat16)
        nc.sync.dma_start(out=wneiT_sb[:], in_=wneiT[:])
        bself_sb = cpool.tile([D, 1], dt.float32)
        nc.sync.dma_start(out=bself_sb[:], in_=bself[:])

        # [128, cp, 2, 128]: row (cp*256 + i*128 + p) of padded h, fp8
        h8r = h8_sb.rearrange("p (cp two m) -> p cp two m", two=2, m=128)

        pa = pagg.tile([128, NPC], dt.float32)
        for cp in range(NCP):
            s = spool.tile([128, 2 * NPC], dt.float8e4)
            nc.sync.dma_start(out=s[:], in_=smat[:, cp * 2 * NPC : (cp + 1) * 2 * NPC])
            sr = s.rearrange("p (two n) -> p two n", two=2)
            for k in range(TPT):
                nc.tensor.matmul(
                    out=pa[:, k * TILE2 : (k + 1) * TILE2],
                    lhsT=h8r[:, cp, :, :],
                    rhs=sr[:, :, k * TILE2 : (k + 1) * TILE2],
                    start=(cp == 0),
                    stop=(cp == NCP - 1),
                    perf_mode=DR,
                )

        for k in range(TPT):
            sl = slice(k * TILE2, (k + 1) * TILE2)
            aggT = apool.tile([128, TILE2], dt.bfloat16)
            nc.vector.tensor_mul(out=aggT[:], in0=pa[:, sl], in1=recip_sb[:, sl])
            po = pout.tile([128, TILE2], dt.float32, space="PSUM")
            nc.tensor.matmul(
                out=po[:], lhsT=wselfT_sb[:], rhs=hT_sb[:, sl], start=True, stop=False
            )
            nc.tensor.matmul(
                out=po[:], lhsT=wneiT_sb[:], rhs=aggT[:], start=False, stop=True
            )
            o = opool.tile([128, TILE2], dt.float16)
            nc.scalar.activation(
                out=o[:],
                in_=po[:],
                func=mybir.ActivationFunctionType.Relu,
                bias=bself_sb[:, :1],
            )
            nc.gpsimd.dma_start(out=outT[:, sl], in_=o[:])

    nc.compile()
    return nc


def _host_prep(h, edge_index, deg):
    import ml_dtypes

    f8 = ml_dtypes.float8_e4m3
    bf16 = ml_dtypes.bfloat16

    src = np.asarray(edge_index[0], dtype=np.int64)
    dst = np.asarray(edge_index[1], dtype=np.int64)
    h = np.asarray(h, dtype=np.float32)
    deg = np.asarray(deg, dtype=np.float32)

    h_pad = np.zeros((N_PAD, D), np.float32)
    h_pad[:N_NODES] = h
    h8_flat = (
        h_pad.astype(f8).reshape(NCP, 2, 128, D).transpose(2, 0, 1, 3).reshape(128, -1)
    )
    h8_flat = np.ascontiguousarray(h8_flat)

    recip = np.zeros(N_PAD, np.float32)
    recip[:N_NODES] = 1.0 / np.maximum(deg, 1.0)

    lut = np.arange(256).astype(np.float32).astype(f8)

    core_of_dst = dst // NPC
    order = np.argsort(core_of_dst, kind="stable")
    src_s, dst_s = src[order], dst[order]
    bounds = np.searchsorted(core_of_dst[order], np.arange(N_CORES + 1))

    per_core = []
    for cc in range(N_CORES):
        lo, hi = bounds[cc], bounds[cc + 1]
        s_u8 = np.zeros((N_PAD, NPC), np.uint8)
        np.add.at(s_u8, (src_s[lo:hi], dst_s[lo:hi] - cc * NPC), 1)
        s8 = lut[s_u8]
        s8 = s8.reshape(NCP, 2, 128, NPC).transpose(2, 0, 1, 3).reshape(128, -1)
        per_core.append(np.ascontiguousarray(s8))

    hT_bf = np.ascontiguousarray(h_pad.T.astype(bf16))
    return h8_flat, per_core, recip, hT_bf


def kernel(h, edge_index, deg, w_self, b_self, w_nei):
    import os

    import ml_dtypes
    from concourse.bass_utils import run_bass_kernel_spmd

    bf16 = ml_dtypes.bfloat16

    h8_flat, per_core, recip, hT_bf = _host_prep(h, edge_index, deg)

    wselfT = np.ascontiguousarray(np.asarray(w_self, dtype=np.float32).T.astype(bf16))
    wneiT = np.ascontiguousarray(np.asarray(w_nei, dtype=np.float32).T.astype(bf16))
    b_col = np.ascontiguousarray(np.asarray(b_self, dtype=np.float32).reshape(D, 1))

    in_maps = []
    for cc in range(N_CORES):
        in_maps.append(
            {
                "h8": h8_flat,
                "smat": per_core[cc],
                "hT": np.ascontiguousarray(hT_bf[:, cc * NPC : (cc + 1) * NPC]),
                "recip": np.ascontiguousarray(
                    recip[cc * NPC : (cc + 1) * NPC].reshape(1, NPC)
                ),
                "wselfT": wselfT,
                "wneiT": wneiT,
                "bself": b_col,
            }
        )

    if "v43" not in _prog_cache:
        _prog_cache["v43"] = _build_program43()
    nc = _prog_cache["v43"]

    trace = bool(int(os.environ.get("GCN_TRACE", "0")))
    res = run_bass_kernel_spmd(nc, in_maps, core_ids=list(range(N_CORES)), trace=trace)
    kernel.last_results = res

    outT = np.concatenate([r["outT"] for r in res.results], axis=1)
    return np.ascontiguousarray(outT[:, :N_NODES].T.astype(np.float32))
